# revision 28
# baseline (speedup 1.0000x reference)
"""Trainium2 Bass kernel for nn_ContrastLoss (supervised-contrastive loss).

Reference computation (B=1024, D=128, C=100, K=32768, N=B+K=33792):
    l   = concat(labels, queue_label.T)          # [N, C]
    w   = labels @ l.T                           # [B, N] shared-class counts
    sim = query @ concat(keys, queue.T).T / T    # [B, N]
    logits = sim - rowmax(sim)
    denom  = sum(exp(logits) * logits_mask, 1)   # logits_mask zeros keys-diag
    loss = -(T/BT) * sqrt(w/max(w)) * (logits - log(denom))

Structure ("recompute", v3):
  * Data-parallel over B: core c owns rows [c*128, (c+1)*128), all N cols.
  * Softmax stabilizer = 1.0 (inputs are L2-normalized), kills rowmax.
  * Self-diagonal handled via host-computed qk_i = q_i . k_i: subtract
    exp((qk-1)/T) from the denominator (no masked pass).
  * Phase A (chunks of 2048): sim matmul (bf16) -> PSUM; ACT Exp reads
    PSUM directly (sole reader) -> bf16 e_scr; the idle DVE row-sums
    e_scr into acc (beats ACT accum_out reads by 3us).  Raw sims are
    NOT evacuated -- phase B re-runs the matmul from the SBUF-resident
    rsim, which deletes the whole DVE cast pass (the old bottleneck).
  * Phase B (chunks of 1024, two double-buffered PSUM pools so the
    matmuls stay OFF the ACT/DVE critical path): w matmul (fp8, exact
    for 0/1 labels) -> Sqrt -> sT; sim matmul again -> psum; one DVE
    scalar_tensor_tensor computes o = (raw - tc) * sT straight from
    PSUM (o = -loss; host negates).
  * Sqrt's scale comes from an AP derived from ln(denom) purely to pin
    the ACT queue order Ln -> Sqrt (avoids ACT-table thrash), and
    output DMAs issue from the idle Pool sequencer so they never queue
    behind input DMAs on SP.
"""

import numpy as np
import ml_dtypes

import concourse.bass as bass
import concourse.mybir as mybir
import concourse.tile as tile
from concourse import bacc
from concourse.bass_utils import run_bass_kernel_spmd

F32 = mybir.dt.float32
BF16 = mybir.dt.bfloat16
FP8 = mybir.dt.float8e4
ALU = mybir.AluOpType
ACTF = mybir.ActivationFunctionType

B, D, C, KQ = 1024, 128, 100, 32768
N = B + KQ                  # 33792 similarity columns
NCORES = 8
ROWS = B // NCORES          # 128 rows per core
STAB = 1.0                  # softmax stabilizer m (raw sim values in [-1, 1])

CHA = 2048                  # phase A steady-state chunk: 4 PSUM banks
# Graduated ramp-in (512/512/1024) so the first Exp starts ~4us earlier,
# then 2048-chunks; N = 33792 = 512+512+1024 + 15*2048 + 1024.
_a_sizes = [512, 512, 1024] + [2048] * 15 + [1024]
assert sum(_a_sizes) == N
ACHUNKS = []
_off = 0
for _s in _a_sizes:
    ACHUNKS.append((_off, _s))
    _off += _s
CHB = 1536                  # phase B output chunk: 3 PSUM banks
BCHUNKS = [(i * CHB, CHB) for i in range(N // CHB)]   # 22 exact chunks
CHW = 512                   # w-matmul / sqrt chunk: 1 PSUM bank


def _build_nc(Tf: float, BTf: float, wmax: float):
    nc = bacc.Bacc("TRN2", target_bir_lowering=False, debug=False,
                   num_devices=NCORES)

    qTb_d = nc.dram_tensor("qTb", [D, ROWS], BF16, kind="ExternalInput")
    labTb_d = nc.dram_tensor("labTb", [C, ROWS], FP8, kind="ExternalInput")
    qk_d = nc.dram_tensor("qk", [ROWS, 1], F32, kind="ExternalInput")
    rsimk_d = nc.dram_tensor("rsimk", [D, B], BF16, kind="ExternalInput")
    rsimq_d = nc.dram_tensor("rsimq", [D, KQ], FP8, kind="ExternalInput")
    rw_d = nc.dram_tensor("rw", [C, N], FP8, kind="ExternalInput")
    out_d = nc.dram_tensor("out", [ROWS, N], BF16, kind="ExternalOutput")

    sq_scale = 1.0 / (BTf * BTf * max(wmax, 1.0))

    with tile.TileContext(nc) as tc:
        with (
            tc.tile_pool(name="const", bufs=1) as const,
            tc.tile_pool(name="escr", bufs=2) as escr_p,
            tc.tile_pool(name="sT", bufs=2) as sT_p,
            tc.tile_pool(name="outp", bufs=3) as outp,
        ):
          with (
            tc.tile_pool(name="psA", bufs=2, space="PSUM") as psA,
          ):
            # ---- resident inputs.  qTb + rsim chunk 0 land first so the
            # first matmul starts early; the rsim tail uses 4 big DMAs to
            # save SP sequencer issue time.  rw issues from the Pool
            # sequencer and is only needed once phase B starts. ------------
            qTb = const.tile([D, ROWS], BF16)
            nc.sync.dma_start(out=qTb[:], in_=qTb_d[:])
            labTb = const.tile([C, ROWS], FP8)
            nc.sync.dma_start(out=labTb[:], in_=labTb_d[:])
            qk = const.tile([ROWS, 1], F32)
            nc.sync.dma_start(out=qk[:], in_=qk_d[:])
            # rsim: bf16 keys block + fp8 queue block (fp8 halves the
            # dominant input stream; quantization adds ~1e-3 rel err).
            # Chunks stream JIT for phase A on the SP queue with rw chunks
            # interleaved behind them (the fp8 diet leaves bandwidth).
            rsimk = const.tile([D, B], BF16)
            rsimq = const.tile([D, KQ], FP8)
            rw = const.tile([C, N], FP8)
            rwch = [(i * 2048, 2048) for i in range(N // 2048)] + (
                [(N - N % 2048, N % 2048)] if N % 2048 else [])
            rw_iter = iter(rwch)
            for k, (base, n) in enumerate(ACHUNKS):
                if base < B:
                    nc.sync.dma_start(out=rsimk[:, base:base + n],
                                      in_=rsimk_d[:, base:base + n])
                else:
                    qb = base - B
                    nc.sync.dma_start(out=rsimq[:, qb:qb + n],
                                      in_=rsimq_d[:, qb:qb + n])
                if k >= 2:
                    try:
                        rb2, rn2 = next(rw_iter)
                        nc.sync.dma_start(out=rw[:, rb2:rb2 + rn2],
                                          in_=rw_d[:, rb2:rb2 + rn2])
                    except StopIteration:
                        pass
            for rb2, rn2 in rw_iter:
                nc.sync.dma_start(out=rw[:, rb2:rb2 + rn2],
                                  in_=rw_d[:, rb2:rb2 + rn2])

            def sim_rhs(off):
                """rsim source for the 512-wide subchunk at global col off."""
                if off < B:
                    return rsimk[:, off:off + 512]
                return rsimq[:, off - B:off - B + 512]

            ebias = const.tile([ROWS, 1], F32)
            nc.vector.memset(ebias, -STAB / Tf)
            zbias = const.tile([ROWS, 1], F32)
            nc.vector.memset(zbias, 0.0)

            # self-diagonal exp runs up front (same table set as phase A's
            # Exp) so the A->B seam only carries Ln + tc.
            eself = const.tile([ROWS, 1], F32)
            nc.scalar.activation(eself[:], qk[:], ACTF.Exp,
                                 bias=ebias[:], scale=1.0 / Tf)

            # int constants for the DVE log2 bit-hack at the seam
            I32 = mybir.dt.int32
            c23 = const.tile([ROWS, 1], I32)
            nc.vector.memset(c23, 23)
            c127 = const.tile([ROWS, 1], I32)
            nc.vector.memset(c127, 127)
            cman = const.tile([ROWS, 1], I32)
            nc.vector.memset(cman, 0x7FFFFF)
            cone = const.tile([ROWS, 1], I32)
            nc.vector.memset(cone, 0x3F800000)

            # ---- phase A: sim matmul -> Exp(PSUM) with rowsum accum ------
            acc = const.tile([ROWS, len(ACHUNKS)], F32)
            for k, (base, n) in enumerate(ACHUNKS):
                ps = psA.tile([ROWS, n], F32, tag="pa")
                for o in range(0, n, 512):
                    nc.tensor.matmul(ps[:, o:o + 512], qTb[:],
                                     sim_rhs(base + o),
                                     start=True, stop=True)
                e_scr = escr_p.tile([ROWS, n], BF16, tag="e")
                nc.scalar.activation(e_scr[:], ps[:], ACTF.Exp,
                                      bias=ebias[:], scale=1.0 / Tf,
                                      accum_out=acc[:, k:k + 1])

            # ---- per-row constant tc = T*ln(denom) + STAB ----------------
            # ln via DVE bit-hack (exponent extract + cubic log2(mantissa)):
            # saves the ACT Ln-table load (1.3us) at the A->B seam.
            dnsum = const.tile([ROWS, 1], F32)
            nc.vector.tensor_reduce(dnsum[:], acc[:], axis=mybir.AxisListType.X,
                                    op=ALU.add)
            # Phase-B Sqrts take their (zero) bias from an AP derived from
            # dnsum: a pure data dependency that keeps the sqrt table load
            # strictly after the last Exp (no ACT-table thrash).
            zbias2 = const.tile([ROWS, 1], F32)
            nc.vector.tensor_scalar(zbias2[:], dnsum[:], 0.0, None,
                                    op0=ALU.mult)
            denom = const.tile([ROWS, 1], F32)
            nc.vector.tensor_sub(denom[:], dnsum[:], eself[:])
            db = denom[:].bitcast(I32)
            e_i = const.tile([ROWS, 1], I32)
            nc.vector.tensor_tensor(e_i[:], db, c23[:],
                                    op=ALU.logical_shift_right)
            nc.vector.tensor_tensor(e_i[:], e_i[:], c127[:], op=ALU.subtract)
            e_f = const.tile([ROWS, 1], F32)
            nc.vector.tensor_copy(out=e_f[:], in_=e_i[:])
            m_i = const.tile([ROWS, 1], I32)
            nc.vector.tensor_tensor(m_i[:], db, cman[:], op=ALU.bitwise_and)
            nc.vector.tensor_tensor(m_i[:], m_i[:], cone[:], op=ALU.bitwise_or)
            m_f = m_i[:].bitcast(F32)
            # log2(m) ~= ((c3*m + c2)*m + c1)*m + c0,  max err 1.4e-3
            LC0, LC1, LC2, LC3 = (-2.13388667, 3.01085106,
                                  -1.02955843, 0.15392466)
            p = const.tile([ROWS, 1], F32)
            nc.vector.tensor_scalar(p[:], m_f, LC3, LC2,
                                    op0=ALU.mult, op1=ALU.add)
            nc.vector.tensor_tensor(p[:], p[:], m_f, op=ALU.mult)
            nc.vector.tensor_scalar(p[:], p[:], 1.0, LC1,
                                    op0=ALU.mult, op1=ALU.add)
            nc.vector.tensor_tensor(p[:], p[:], m_f, op=ALU.mult)
            nc.vector.tensor_scalar(p[:], p[:], 1.0, LC0,
                                    op0=ALU.mult, op1=ALU.add)
            # tc = T*ln2*(e + log2(m)) + STAB
            lnd = const.tile([ROWS, 1], F32)
            nc.vector.tensor_add(lnd[:], e_f[:], p[:])
            tc_row = const.tile([ROWS, 1], F32)
            nc.vector.tensor_scalar(tc_row[:], lnd[:], Tf * 0.6931471805599453,
                                    STAB, op0=ALU.mult, op1=ALU.add)

          with (
            tc.tile_pool(name="psW", bufs=2, space="PSUM") as psW,
            tc.tile_pool(name="psB", bufs=2, space="PSUM") as psB,
          ):
            # ---- phase B: w matmul -> sT; sim re-matmul -> fused output --
            # Sqrt runs on 512-col psW tiles (1 bank, imm scale except the
            # first, which takes the sq_ap AP to pin Ln -> Sqrt table
            # order); the stt consumes 1536-col psB tiles.
            for k, (base, n) in enumerate(BCHUNKS):
                sT = sT_p.tile([ROWS, n], BF16, tag="s")
                for o in range(0, n, CHW):
                    psw = psW.tile([ROWS, CHW], F32, tag="pw")
                    nc.tensor.matmul(psw[:], labTb[:],
                                     rw[:, base + o:base + o + CHW],
                                     start=True, stop=True)
                    nc.scalar.activation(sT[:, o:o + CHW], psw[:],
                                         ACTF.Sqrt, bias=zbias2[:],
                                         scale=sq_scale)
                psb = psB.tile([ROWS, n], F32, tag="pb")
                for o in range(0, n, 512):
                    nc.tensor.matmul(psb[:, o:o + 512], qTb[:],
                                     sim_rhs(base + o),
                                     start=True, stop=True)
                o_t = outp.tile([ROWS, n], BF16, tag="o")
                # o = (raw - tc) * sT = -loss; negated on the host.
                nc.vector.scalar_tensor_tensor(
                    o_t[:], psb[:], tc_row[:], sT[:],
                    op0=ALU.subtract, op1=ALU.mult,
                )
                # Early output chunks ride the Pool SWDGE queue; later ones
                # the SP queue (idle once the inputs have drained) so the
                # write stream finishes with the compute instead of
                # dribbling out of one ~160GB/s queue afterwards.
                oeng = nc.gpsimd if k < 11 else nc.sync
                oeng.dma_start(out=out_d[:, base:base + n], in_=o_t[:])
    nc.compile()
    return nc


def _host_prep(query, keys, labels, queue, queue_label):
    bf16 = ml_dtypes.bfloat16
    fp8 = ml_dtypes.float8_e4m3
    query = np.asarray(query, np.float32)
    keys = np.asarray(keys, np.float32)
    labels = np.asarray(labels, np.float32)
    queue = np.asarray(queue, np.float32)
    queue_label = np.asarray(queue_label, np.float32)

    qT = query.T                                        # [D, B]
    labT = labels.T                                     # [C, B]
    qk = (query * keys).sum(axis=1, keepdims=True).astype(np.float32)
    rsimk = np.ascontiguousarray(keys.T).astype(bf16)           # [D, B]
    rsimq = np.ascontiguousarray(queue).astype(fp8)             # [D, KQ]
    rw = np.ascontiguousarray(
        np.concatenate([labT, queue_label], axis=1)).astype(fp8)  # [C, N]

    in_maps = []
    for c in range(NCORES):
        blk = slice(c * ROWS, (c + 1) * ROWS)
        in_maps.append({
            "qTb": np.ascontiguousarray(qT[:, blk]).astype(bf16),
            "labTb": np.ascontiguousarray(labT[:, blk]).astype(fp8),
            "qk": np.ascontiguousarray(qk[blk]),
            "rsimk": rsimk,
            "rsimq": rsimq,
            "rw": rw,
        })
    return in_maps


def _gather_output(results):
    out = np.empty((B, N), np.float32)
    for c in range(NCORES):
        out[c * ROWS:(c + 1) * ROWS, :] = -results[c]["out"].astype(np.float32)
    return out


def kernel(query, keys, labels, queue, queue_label, K, T, BT, **_unused):
    Tf = float(np.asarray(T))
    BTf = float(np.asarray(BT))
    labels = np.asarray(labels, np.float32)
    wmax = float(labels.sum(axis=1).max())
    nc = _build_nc(Tf, BTf, wmax)
    in_maps = _host_prep(query, keys, labels, queue, queue_label)
    res = run_bass_kernel_spmd(nc, in_maps, list(range(NCORES)))
    return _gather_output(res.results)


# Re-usable entry for test.py: returns (output, BassKernelResults) so the
# harness there can pull exec_time_ns / profile out of a traced run.
def kernel_traced(query, keys, labels, queue, queue_label, K, T, BT,
                  trace=False, **run_kwargs):
    Tf = float(np.asarray(T))
    BTf = float(np.asarray(BT))
    labels = np.asarray(labels, np.float32)
    wmax = float(labels.sum(axis=1).max())
    nc = _build_nc(Tf, BTf, wmax)
    in_maps = _host_prep(query, keys, labels, queue, queue_label)
    res = run_bass_kernel_spmd(nc, in_maps, list(range(NCORES)),
                               trace=trace, **run_kwargs)
    return _gather_output(res.results), res


# revision 29
# speedup vs baseline: 1.0151x; 1.0151x over previous
"""Trainium2 Bass kernel for nn_ContrastLoss (supervised-contrastive loss).

Reference computation (B=1024, D=128, C=100, K=32768, N=B+K=33792):
    l   = concat(labels, queue_label.T)          # [N, C]
    w   = labels @ l.T                           # [B, N] shared-class counts
    sim = query @ concat(keys, queue.T).T / T    # [B, N]
    logits = sim - rowmax(sim)
    denom  = sum(exp(logits) * logits_mask, 1)   # logits_mask zeros keys-diag
    loss = -(T/BT) * sqrt(w/max(w)) * (logits - log(denom))

Structure ("recompute", v3):
  * Data-parallel over B: core c owns rows [c*128, (c+1)*128), all N cols.
  * Softmax stabilizer = 1.0 (inputs are L2-normalized), kills rowmax.
  * Self-diagonal handled via host-computed qk_i = q_i . k_i: subtract
    exp((qk-1)/T) from the denominator (no masked pass).
  * Phase A (chunks of 2048): sim matmul (bf16) -> PSUM; ACT Exp reads
    PSUM directly (sole reader) -> bf16 e_scr; the idle DVE row-sums
    e_scr into acc (beats ACT accum_out reads by 3us).  Raw sims are
    NOT evacuated -- phase B re-runs the matmul from the SBUF-resident
    rsim, which deletes the whole DVE cast pass (the old bottleneck).
  * Phase B (chunks of 1024, two double-buffered PSUM pools so the
    matmuls stay OFF the ACT/DVE critical path): w matmul (fp8, exact
    for 0/1 labels) -> Sqrt -> sT; sim matmul again -> psum; one DVE
    scalar_tensor_tensor computes o = (raw - tc) * sT straight from
    PSUM (o = -loss; host negates).
  * Sqrt's scale comes from an AP derived from ln(denom) purely to pin
    the ACT queue order Ln -> Sqrt (avoids ACT-table thrash), and
    output DMAs issue from the idle Pool sequencer so they never queue
    behind input DMAs on SP.
"""

import numpy as np
import ml_dtypes

import concourse.bass as bass
import concourse.mybir as mybir
import concourse.tile as tile
from concourse import bacc
from concourse.bass_utils import run_bass_kernel_spmd

F32 = mybir.dt.float32
BF16 = mybir.dt.bfloat16
FP8 = mybir.dt.float8e4
ALU = mybir.AluOpType
ACTF = mybir.ActivationFunctionType

B, D, C, KQ = 1024, 128, 100, 32768
N = B + KQ                  # 33792 similarity columns
NCORES = 8
ROWS = B // NCORES          # 128 rows per core
STAB = 1.0                  # softmax stabilizer m (raw sim values in [-1, 1])

CHA = 2048                  # phase A steady-state chunk: 4 PSUM banks
# Graduated ramp-in (512/512/1024) so the first Exp starts ~4us earlier,
# then 2048-chunks; N = 33792 = 512+512+1024 + 15*2048 + 1024.
_a_sizes = [512, 512, 1024] + [2048] * 15 + [1024]
assert sum(_a_sizes) == N
ACHUNKS = []
_off = 0
for _s in _a_sizes:
    ACHUNKS.append((_off, _s))
    _off += _s
CHB = 1536                  # phase B output chunk: 3 PSUM banks
BCHUNKS = [(i * CHB, CHB) for i in range(N // CHB)]   # 22 exact chunks
CHW = 512                   # w-matmul / sqrt chunk: 1 PSUM bank


def _build_nc(Tf: float, BTf: float, wmax: float):
    nc = bacc.Bacc("TRN2", target_bir_lowering=False, debug=False,
                   num_devices=NCORES)

    qTb_d = nc.dram_tensor("qTb", [D, ROWS], BF16, kind="ExternalInput")
    labTb_d = nc.dram_tensor("labTb", [C, ROWS], FP8, kind="ExternalInput")
    qk_d = nc.dram_tensor("qk", [ROWS, 1], F32, kind="ExternalInput")
    rsimk_d = nc.dram_tensor("rsimk", [D, B], BF16, kind="ExternalInput")
    rsimq_d = nc.dram_tensor("rsimq", [D, KQ], FP8, kind="ExternalInput")
    rw_d = nc.dram_tensor("rw", [C, N], FP8, kind="ExternalInput")
    out_d = nc.dram_tensor("out", [ROWS, N], BF16, kind="ExternalOutput")

    sq_scale = 1.0 / (BTf * BTf * max(wmax, 1.0))

    with tile.TileContext(nc) as tc:
        with (
            tc.tile_pool(name="const", bufs=1) as const,
            tc.tile_pool(name="escr", bufs=2) as escr_p,
            tc.tile_pool(name="sT", bufs=2) as sT_p,
            tc.tile_pool(name="outp", bufs=3) as outp,
        ):
          with (
            tc.tile_pool(name="psA", bufs=2, space="PSUM") as psA,
          ):
            # ---- resident inputs.  qTb + rsim chunk 0 land first so the
            # first matmul starts early; the rsim tail uses 4 big DMAs to
            # save SP sequencer issue time.  rw issues from the Pool
            # sequencer and is only needed once phase B starts. ------------
            qTb = const.tile([D, ROWS], BF16)
            nc.sync.dma_start(out=qTb[:], in_=qTb_d[:])
            labTb = const.tile([C, ROWS], FP8)
            qk = const.tile([ROWS, 1], F32)
            # rsim: bf16 keys block + fp8 queue block (fp8 halves the
            # dominant input stream; quantization adds ~1e-3 rel err).
            # Chunks stream JIT for phase A on the SP queue with rw chunks
            # interleaved behind them (the fp8 diet leaves bandwidth).
            rsimk = const.tile([D, B], BF16)
            rsimq = const.tile([D, KQ], FP8)
            rw = const.tile([C, N], FP8)
            rwch = [(i * 2048, 2048) for i in range(N // 2048)] + (
                [(N - N % 2048, N % 2048)] if N % 2048 else [])
            rw_iter = iter(rwch)
            for k, (base, n) in enumerate(ACHUNKS):
                if base < B:
                    nc.sync.dma_start(out=rsimk[:, base:base + n],
                                      in_=rsimk_d[:, base:base + n])
                else:
                    qb = base - B
                    nc.sync.dma_start(out=rsimq[:, qb:qb + n],
                                      in_=rsimq_d[:, qb:qb + n])
                if k == 3:
                    nc.sync.dma_start(out=labTb[:], in_=labTb_d[:])
                    nc.sync.dma_start(out=qk[:], in_=qk_d[:])
                if k >= 2:
                    try:
                        rb2, rn2 = next(rw_iter)
                        nc.sync.dma_start(out=rw[:, rb2:rb2 + rn2],
                                          in_=rw_d[:, rb2:rb2 + rn2])
                    except StopIteration:
                        pass
            for rb2, rn2 in rw_iter:
                nc.sync.dma_start(out=rw[:, rb2:rb2 + rn2],
                                  in_=rw_d[:, rb2:rb2 + rn2])

            def sim_rhs(off):
                """rsim source for the 512-wide subchunk at global col off."""
                if off < B:
                    return rsimk[:, off:off + 512]
                return rsimq[:, off - B:off - B + 512]

            ebias = const.tile([ROWS, 1], F32)
            nc.vector.memset(ebias, -STAB / Tf)
            zbias = const.tile([ROWS, 1], F32)
            nc.vector.memset(zbias, 0.0)

            # self-diagonal exp runs up front (same table set as phase A's
            # Exp) so the A->B seam only carries Ln + tc.
            eself = const.tile([ROWS, 1], F32)
            nc.scalar.activation(eself[:], qk[:], ACTF.Exp,
                                 bias=ebias[:], scale=1.0 / Tf)

            # int constants for the DVE log2 bit-hack at the seam
            I32 = mybir.dt.int32
            c23 = const.tile([ROWS, 1], I32)
            nc.vector.memset(c23, 23)
            c127 = const.tile([ROWS, 1], I32)
            nc.vector.memset(c127, 127)
            cman = const.tile([ROWS, 1], I32)
            nc.vector.memset(cman, 0x7FFFFF)
            cone = const.tile([ROWS, 1], I32)
            nc.vector.memset(cone, 0x3F800000)

            # ---- phase A: sim matmul -> Exp(PSUM) with rowsum accum ------
            acc = const.tile([ROWS, len(ACHUNKS)], F32)
            for k, (base, n) in enumerate(ACHUNKS):
                ps = psA.tile([ROWS, n], F32, tag="pa")
                for o in range(0, n, 512):
                    nc.tensor.matmul(ps[:, o:o + 512], qTb[:],
                                     sim_rhs(base + o),
                                     start=True, stop=True)
                e_scr = escr_p.tile([ROWS, n], BF16, tag="e")
                nc.scalar.activation(e_scr[:], ps[:], ACTF.Exp,
                                      bias=ebias[:], scale=1.0 / Tf,
                                      accum_out=acc[:, k:k + 1])

            # ---- per-row constant tc = T*ln(denom) + STAB ----------------
            # ln via DVE bit-hack (exponent extract + cubic log2(mantissa)):
            # saves the ACT Ln-table load (1.3us) at the A->B seam.
            dnsum = const.tile([ROWS, 1], F32)
            nc.vector.tensor_reduce(dnsum[:], acc[:], axis=mybir.AxisListType.X,
                                    op=ALU.add)
            # Phase-B Sqrts take their (zero) bias from an AP derived from
            # dnsum: a pure data dependency that keeps the sqrt table load
            # strictly after the last Exp (no ACT-table thrash).
            zbias2 = const.tile([ROWS, 1], F32)
            nc.vector.tensor_scalar(zbias2[:], dnsum[:], 0.0, None,
                                    op0=ALU.mult)
            denom = const.tile([ROWS, 1], F32)
            nc.vector.tensor_sub(denom[:], dnsum[:], eself[:])
            db = denom[:].bitcast(I32)
            e_i = const.tile([ROWS, 1], I32)
            nc.vector.tensor_tensor(e_i[:], db, c23[:],
                                    op=ALU.logical_shift_right)
            nc.vector.tensor_tensor(e_i[:], e_i[:], c127[:], op=ALU.subtract)
            e_f = const.tile([ROWS, 1], F32)
            nc.vector.tensor_copy(out=e_f[:], in_=e_i[:])
            m_i = const.tile([ROWS, 1], I32)
            nc.vector.tensor_tensor(m_i[:], db, cman[:], op=ALU.bitwise_and)
            nc.vector.tensor_tensor(m_i[:], m_i[:], cone[:], op=ALU.bitwise_or)
            m_f = m_i[:].bitcast(F32)
            # log2(m) ~= ((c3*m + c2)*m + c1)*m + c0,  max err 1.4e-3
            LC0, LC1, LC2, LC3 = (-2.13388667, 3.01085106,
                                  -1.02955843, 0.15392466)
            p = const.tile([ROWS, 1], F32)
            nc.vector.tensor_scalar(p[:], m_f, LC3, LC2,
                                    op0=ALU.mult, op1=ALU.add)
            nc.vector.tensor_tensor(p[:], p[:], m_f, op=ALU.mult)
            nc.vector.tensor_scalar(p[:], p[:], 1.0, LC1,
                                    op0=ALU.mult, op1=ALU.add)
            nc.vector.tensor_tensor(p[:], p[:], m_f, op=ALU.mult)
            nc.vector.tensor_scalar(p[:], p[:], 1.0, LC0,
                                    op0=ALU.mult, op1=ALU.add)
            # tc = T*ln2*(e + log2(m)) + STAB
            lnd = const.tile([ROWS, 1], F32)
            nc.vector.tensor_add(lnd[:], e_f[:], p[:])
            tc_row = const.tile([ROWS, 1], F32)
            nc.vector.tensor_scalar(tc_row[:], lnd[:], Tf * 0.6931471805599453,
                                    STAB, op0=ALU.mult, op1=ALU.add)

          with (
            tc.tile_pool(name="psW", bufs=2, space="PSUM") as psW,
            tc.tile_pool(name="psB", bufs=2, space="PSUM") as psB,
          ):
            # ---- phase B: w matmul -> sT; sim re-matmul -> fused output --
            # Sqrt runs on 512-col psW tiles (1 bank, imm scale except the
            # first, which takes the sq_ap AP to pin Ln -> Sqrt table
            # order); the stt consumes 1536-col psB tiles.
            for k, (base, n) in enumerate(BCHUNKS):
                sT = sT_p.tile([ROWS, n], BF16, tag="s")
                for o in range(0, n, CHW):
                    psw = psW.tile([ROWS, CHW], F32, tag="pw")
                    nc.tensor.matmul(psw[:], labTb[:],
                                     rw[:, base + o:base + o + CHW],
                                     start=True, stop=True)
                    nc.scalar.activation(sT[:, o:o + CHW], psw[:],
                                         ACTF.Sqrt, bias=zbias2[:],
                                         scale=sq_scale)
                psb = psB.tile([ROWS, n], F32, tag="pb")
                for o in range(0, n, 512):
                    nc.tensor.matmul(psb[:, o:o + 512], qTb[:],
                                     sim_rhs(base + o),
                                     start=True, stop=True)
                o_t = outp.tile([ROWS, n], BF16, tag="o")
                # o = (raw - tc) * sT = -loss; negated on the host.
                nc.vector.scalar_tensor_tensor(
                    o_t[:], psb[:], tc_row[:], sT[:],
                    op0=ALU.subtract, op1=ALU.mult,
                )
                # Early output chunks ride the Pool SWDGE queue; later ones
                # the SP queue (idle once the inputs have drained) so the
                # write stream finishes with the compute instead of
                # dribbling out of one ~160GB/s queue afterwards.
                oeng = nc.gpsimd if k % 2 == 0 else nc.sync
                oeng.dma_start(out=out_d[:, base:base + n], in_=o_t[:])
    nc.compile()
    return nc


def _host_prep(query, keys, labels, queue, queue_label):
    bf16 = ml_dtypes.bfloat16
    fp8 = ml_dtypes.float8_e4m3
    query = np.asarray(query, np.float32)
    keys = np.asarray(keys, np.float32)
    labels = np.asarray(labels, np.float32)
    queue = np.asarray(queue, np.float32)
    queue_label = np.asarray(queue_label, np.float32)

    qT = query.T                                        # [D, B]
    labT = labels.T                                     # [C, B]
    qk = (query * keys).sum(axis=1, keepdims=True).astype(np.float32)
    rsimk = np.ascontiguousarray(keys.T).astype(bf16)           # [D, B]
    rsimq = np.ascontiguousarray(queue).astype(fp8)             # [D, KQ]
    rw = np.ascontiguousarray(
        np.concatenate([labT, queue_label], axis=1)).astype(fp8)  # [C, N]

    in_maps = []
    for c in range(NCORES):
        blk = slice(c * ROWS, (c + 1) * ROWS)
        in_maps.append({
            "qTb": np.ascontiguousarray(qT[:, blk]).astype(bf16),
            "labTb": np.ascontiguousarray(labT[:, blk]).astype(fp8),
            "qk": np.ascontiguousarray(qk[blk]),
            "rsimk": rsimk,
            "rsimq": rsimq,
            "rw": rw,
        })
    return in_maps


def _gather_output(results):
    out = np.empty((B, N), np.float32)
    for c in range(NCORES):
        out[c * ROWS:(c + 1) * ROWS, :] = -results[c]["out"].astype(np.float32)
    return out


def kernel(query, keys, labels, queue, queue_label, K, T, BT, **_unused):
    Tf = float(np.asarray(T))
    BTf = float(np.asarray(BT))
    labels = np.asarray(labels, np.float32)
    wmax = float(labels.sum(axis=1).max())
    nc = _build_nc(Tf, BTf, wmax)
    in_maps = _host_prep(query, keys, labels, queue, queue_label)
    res = run_bass_kernel_spmd(nc, in_maps, list(range(NCORES)))
    return _gather_output(res.results)


# Re-usable entry for test.py: returns (output, BassKernelResults) so the
# harness there can pull exec_time_ns / profile out of a traced run.
def kernel_traced(query, keys, labels, queue, queue_label, K, T, BT,
                  trace=False, **run_kwargs):
    Tf = float(np.asarray(T))
    BTf = float(np.asarray(BT))
    labels = np.asarray(labels, np.float32)
    wmax = float(labels.sum(axis=1).max())
    nc = _build_nc(Tf, BTf, wmax)
    in_maps = _host_prep(query, keys, labels, queue, queue_label)
    res = run_bass_kernel_spmd(nc, in_maps, list(range(NCORES)),
                               trace=trace, **run_kwargs)
    return _gather_output(res.results), res
